# revision 4
# baseline (speedup 1.0000x reference)
"""Trainium2 Bass kernel for nn_Euclidian (segment_reduce):

    counts/centers = segment mean of feat by label (C=100 classes)
    out[i] = || feat[i] - centers[label[i]] ||_2

Strategy (8 NeuronCores, data-parallel over N):
  pass 1: per 128-sample tile, onehot[128,100] = (iota == label); PSUM
          accumulate centers_sum[100,256] += onehot.T @ feat  (PE, f32r)
  AllReduce[100,256] across the 8 cores (tiny); centers = sums * (1/count)
          (1/count precomputed host-side from labels alone)
  pass 2: G[128,256] = onehotT.T @ centers gathers each sample's center row
          on the PE (no HBM gather traffic); onehotT built by broadcasting
          labels across partitions with a K=1 matmul + is_equal.
          dist = sqrt(sum((feat-G)^2)) via DVE subtract + ACT square-accum.

feat is read from HBM exactly twice — memory roofline.
"""

import numpy as np

import concourse.mybir as mybir
import concourse.tile as tile
from concourse import bacc
from concourse.bass_utils import run_bass_kernel_spmd

F32 = mybir.dt.float32
F32R = mybir.dt.float32r
I32 = mybir.dt.int32

P = 128  # partitions / samples per tile
C = 100  # num classes
D = 256  # feature dim

N_FULL = 500000
N_CORES = 8
NS = N_FULL // N_CORES  # 62500 samples per core
GROUP = 8  # tiles per feat DMA group


def _group_sizes(np_pad):
    """Split np_pad samples into groups of GROUP tiles (+ remainder tiles)."""
    n_tiles = np_pad // P
    groups = []
    t = 0
    while t < n_tiles:
        g = min(GROUP, n_tiles - t)
        groups.append(g)
        t += g
    return groups


def build(np_pad, num_devices=N_CORES):
    """Build the per-core SPMD program for np_pad (multiple of 128) samples."""
    assert np_pad % P == 0
    groups = _group_sizes(np_pad)

    nc = bacc.Bacc(
        "TRN2",
        target_bir_lowering=False,
        debug=False,
        enable_asserts=True,
        num_devices=num_devices,
    )

    feat_d = nc.dram_tensor("feat", [np_pad, D], F32, kind="ExternalInput")
    labp_d = nc.dram_tensor("labp", [np_pad], F32, kind="ExternalInput")  # p-major per group
    labf_d = nc.dram_tensor("labf", [np_pad], F32, kind="ExternalInput")  # flat
    crec_d = nc.dram_tensor("crec", [C, 1], F32, kind="ExternalInput")  # 1/max(count,1)
    out_d = nc.dram_tensor("given", [np_pad], F32, kind="ExternalOutput")

    with tile.TileContext(nc) as tc:
        import contextlib

        with contextlib.ExitStack() as ctx:
            const = ctx.enter_context(tc.tile_pool(name="const", bufs=1))
            sb1 = ctx.enter_context(tc.tile_pool(name="sb1", bufs=3))
            oh1 = ctx.enter_context(tc.tile_pool(name="oh1", bufs=4))
            dram = ctx.enter_context(tc.tile_pool(name="dram", bufs=1, space="DRAM"))

            # ---------------- constants ----------------
            iota_i = const.tile([P, C], I32)
            nc.gpsimd.iota(iota_i[:], pattern=[[1, C]], base=0, channel_multiplier=0)
            iota_row = const.tile([P, C], F32)
            nc.vector.tensor_copy(iota_row[:], iota_i[:])

            iotac_i = const.tile([C, 1], I32)
            nc.gpsimd.iota(iotac_i[:], pattern=[[0, 1]], base=0, channel_multiplier=1)
            iota_col = const.tile([C, 1], F32)
            nc.vector.tensor_copy(iota_col[:], iotac_i[:])

            ones_f = const.tile([1, C], F32)
            nc.vector.memset(ones_f[:1, :], 1.0)
            ones_row = const.tile([1, C], F32R)
            nc.vector.tensor_copy(ones_row[:1, :], ones_f[:1, :])

            crec_sb = const.tile([C, 1], F32)
            nc.sync.dma_start(out=crec_sb[:], in_=crec_d[:, :])

            # ---------------- pass 1: local segment sums ----------------
            with tc.tile_pool(name="ps1", bufs=1, space="PSUM") as ps1:
                acc_ps = ps1.tile([C, D], F32, space="PSUM")
                n_tiles_total = np_pad // P
                ti = 0
                off = 0
                for g in groups:
                    w = g * P  # samples in group
                    feat_g = sb1.tile([P, GROUP * D], F32R, tag="feat1")
                    # partition p <- sample row off + t*128 + p, free (t,d)
                    nc.sync.dma_start(
                        out=feat_g[:, : g * D].rearrange("p (t d) -> p t d", d=D),
                        in_=feat_d[off : off + w, :]
                        .rearrange("(t p) d -> p t d", p=P)
                        .bitcast(F32R),
                    )
                    labp_g = sb1.tile([P, GROUP], F32, tag="labp")
                    nc.sync.dma_start(
                        out=labp_g[:, :g],
                        in_=labp_d[off : off + w].rearrange("(p t) -> p t", p=P),
                    )
                    for t in range(g):
                        onehot = oh1.tile([P, C], F32R, tag="oh")
                        nc.vector.tensor_scalar(
                            out=onehot[:],
                            in0=iota_row[:],
                            scalar1=labp_g[:, t : t + 1],
                            scalar2=None,
                            op0=mybir.AluOpType.is_equal,
                        )
                        nc.tensor.matmul(
                            acc_ps[:],
                            lhsT=onehot[:],
                            rhs=feat_g[:, t * D : (t + 1) * D],
                            start=(ti == 0),
                            stop=(ti == n_tiles_total - 1),
                        )
                        ti += 1
                    off += w

                # ---------------- all-reduce of [C, D] sums ----------------
                sums_sb = const.tile([C, D], F32)
                nc.vector.tensor_copy(sums_sb[:], acc_ps[:])

            cc_in = dram.tile([C, D], F32)
            cc_out = dram.tile([C, D], F32)
            nc.sync.dma_start(out=cc_in[:], in_=sums_sb[:])
            if num_devices > 1:
                nc.gpsimd.collective_compute(
                    "AllReduce",
                    mybir.AluOpType.add,
                    replica_groups=[list(range(num_devices))],
                    ins=[cc_in.opt()],
                    outs=[cc_out.opt()],
                )
                gsrc = cc_out
            else:
                gsrc = cc_in
            gsums_sb = const.tile([C, D], F32)
            nc.sync.dma_start(out=gsums_sb[:], in_=gsrc[:])

            # centers = gsums * (1/count), produced as f32r for the PE gather
            centers_r = const.tile([C, D], F32R)
            nc.vector.tensor_scalar(
                out=centers_r[:],
                in0=gsums_sb[:],
                scalar1=crec_sb[:, :1],
                scalar2=None,
                op0=mybir.AluOpType.mult,
            )

            # ---------------- pass 2: distances ----------------
            with (
                tc.tile_pool(name="ps_lb", bufs=2, space="PSUM") as ps_lb,
                tc.tile_pool(name="ps_g", bufs=4, space="PSUM") as ps_g,
                tc.tile_pool(name="sb2", bufs=3) as sb2,
            ):
                off = 0
                for g in groups:
                    w = g * P
                    feat_g = sb2.tile([P, GROUP * D], F32, tag="feat2")
                    nc.sync.dma_start(
                        out=feat_g[:, : g * D].rearrange("p (t d) -> p t d", d=D),
                        in_=feat_d[off : off + w, :].rearrange("(t p) d -> p t d", p=P),
                    )
                    labf_g = sb2.tile([1, GROUP * P], F32R, tag="labf")
                    nc.sync.dma_start(
                        out=labf_g[:1, :w],
                        in_=labf_d[None, off : off + w].bitcast(F32R),
                    )
                    # onehotT for the whole group, built 512 samples at a time
                    oht_g = sb2.tile([C, GROUP * P], F32R, tag="oht")
                    for h in range(0, w, 512):
                        hw = min(512, w - h)
                        lb_ps = ps_lb.tile([C, 512], F32, space="PSUM", tag="lb")
                        nc.tensor.matmul(
                            lb_ps[:, :hw],
                            lhsT=ones_row[:1, :],
                            rhs=labf_g[:1, h : h + hw],
                            start=True,
                            stop=True,
                        )
                        nc.vector.tensor_scalar(
                            out=oht_g[:, h : h + hw],
                            in0=lb_ps[:, :hw],
                            scalar1=iota_col[:, :1],
                            scalar2=None,
                            op0=mybir.AluOpType.is_equal,
                        )
                    res_g = sb2.tile([P, GROUP], F32, tag="res")
                    for t in range(g):
                        g_ps = ps_g.tile([P, D], F32, space="PSUM", tag="g")
                        nc.tensor.matmul(
                            g_ps[:],
                            lhsT=oht_g[:, t * P : (t + 1) * P],
                            rhs=centers_r[:],
                            start=True,
                            stop=True,
                        )
                        diff = sb2.tile([P, D], F32, tag="diff")
                        nc.vector.tensor_tensor(
                            out=diff[:],
                            in0=feat_g[:, t * D : (t + 1) * D],
                            in1=g_ps[:],
                            op=mybir.AluOpType.subtract,
                        )
                        nc.scalar.activation(
                            out=diff[:],
                            in_=diff[:],
                            func=mybir.ActivationFunctionType.Square,
                            accum_out=res_g[:, t : t + 1],
                        )
                    nc.scalar.activation(
                        out=res_g[:, :g],
                        in_=res_g[:, :g],
                        func=mybir.ActivationFunctionType.Sqrt,
                    )
                    # out[off + t*128 + p] = res_g[p, t]
                    nc.sync.dma_start(
                        out=out_d[off : off + w].rearrange("(t p) -> p t", p=P),
                        in_=res_g[:, :g],
                    )
                    off += w

    nc.compile()
    return nc


def build_nop(num_devices=N_CORES):
    """Minimal kernel (copy one tile) to measure the dispatch floor."""
    nc = bacc.Bacc(
        "TRN2",
        target_bir_lowering=False,
        debug=False,
        enable_asserts=True,
        num_devices=num_devices,
    )
    x_d = nc.dram_tensor("x", [P, P], F32, kind="ExternalInput")
    y_d = nc.dram_tensor("y", [P, P], F32, kind="ExternalOutput")
    with tile.TileContext(nc) as tc:
        with tc.tile_pool(name="sb", bufs=1) as sb:
            t = sb.tile([P, P], F32)
            nc.sync.dma_start(out=t[:], in_=x_d[:, :])
            nc.sync.dma_start(out=y_d[:, :], in_=t[:])
    nc.compile()
    return nc


def _prep_core_inputs(feat_c, lab_c, crec, np_pad):
    """Host-side shard prep: pad + layout labels; all f32."""
    ns = feat_c.shape[0]
    fpad = np.zeros((np_pad, D), dtype=np.float32)
    fpad[:ns] = feat_c
    lab = np.full((np_pad,), float(C), dtype=np.float32)  # pad label = C -> no class
    lab[:ns] = lab_c.astype(np.float32)
    # labp: per group, [128, g] stored p-major: labp[off + p*g + t] = lab[off + t*128 + p]
    labp = np.empty_like(lab)
    off = 0
    for g in _group_sizes(np_pad):
        w = g * P
        labp[off : off + w] = lab[off : off + w].reshape(g, P).T.reshape(-1)
        off += w
    return {"feat": fpad, "labp": labp, "labf": lab, "crec": crec}


_CACHE = {}


def _get_nc(np_pad, num_devices):
    key = (np_pad, num_devices)
    if key not in _CACHE:
        _CACHE[key] = build(np_pad, num_devices)
    return _CACHE[key]


def run(feat, label, np_pad=None, num_devices=N_CORES, trace=False):
    n = feat.shape[0]
    ns = n // num_devices
    if np_pad is None:
        np_pad = ((ns + P - 1) // P) * P
    nc = _get_nc(np_pad, num_devices)

    cnt = np.bincount(label.astype(np.int64), minlength=C)[:C]
    crec = (1.0 / np.maximum(cnt, 1)).astype(np.float32)[:, None]

    in_maps = [
        _prep_core_inputs(
            feat[c * ns : (c + 1) * ns], label[c * ns : (c + 1) * ns], crec, np_pad
        )
        for c in range(num_devices)
    ]
    res = run_bass_kernel_spmd(
        nc, in_maps, core_ids=list(range(num_devices)), trace=trace
    )
    out = np.concatenate([res.results[c]["given"][:ns] for c in range(num_devices)])
    return out, res


def kernel(feat, label):
    feat = np.asarray(feat, dtype=np.float32)
    label = np.asarray(label)
    out, _ = run(feat, label)
    return out.astype(np.float32)
